# revision 1
# baseline (speedup 1.0000x reference)
"""GQA transformer block on 8 TRN2 cores.

Sharding: core = (b, k) with b = batch (2), k = kv-head (4).
Each core computes LN1(tokens[b]), projects Q (its 4 q-heads), K/V (its kv
head), does causal attention in S^T orientation (keys on partitions, queries
on the free dim) with unnormalized exp-scores plus a ones-column to produce
softmax denominators from the same matmul, then multiplies by its Wo row-slab
to get a partial [N, D] output. A 4-core ReduceScatter sums the partials and
hands each core a 512-row slab for residual + LN2.

All heavy matmuls run as float32r (1 cycle/row when free >= 256).
"""

import os
import sys
from contextlib import ExitStack

for _p in ("/opt/trn_rl_repo", "/root/.axon_site/_ro/trn_rl_repo"):
    if os.path.isdir(_p) and _p not in sys.path:
        sys.path.insert(0, _p)

import numpy as np

import concourse.bass as bass
import concourse.bacc as bacc
import concourse.tile as tile
from concourse import mybir
from concourse.bass_utils import run_bass_kernel_spmd
from concourse.masks import make_identity

B, N, D = 2, 2048, 1024
HQ, HKV, HD = 16, 4, 64
G = HQ // HKV  # q heads per kv head (= per core)
EPS = 1e-5
P = 128
NT = N // P  # 16 token tiles
DC = D // P  # 8 d-chunks
SLAB = N // 4  # 512 rows per core after reduce-scatter
ST = SLAB // P  # 4 token tiles per slab
F32 = mybir.dt.float32
F32R = mybir.dt.float32r
FD = 512  # matmul free-dim chunk (one PSUM bank)
NC_ = N // FD  # 4 free chunks over queries
RG = [[0, 1, 2, 3], [4, 5, 6, 7]]
AF = mybir.ActivationFunctionType
ALU = mybir.AluOpType
SCALE = 1.0 / np.sqrt(HD)


def _ln_stats(nc, pool, x_ap, eps_tile, p=P):
    """mean/rstd over the free dim (len 1024) of x_ap [p, 1024]."""
    stats = pool.tile([p, 2, nc.vector.BN_STATS_DIM], F32, tag="bst")
    xg = x_ap.rearrange("p (s f) -> p s f", s=2)
    for s in range(2):
        nc.vector.bn_stats(out=stats[:, s, :], in_=xg[:, s, :])
    mv = pool.tile([p, nc.vector.BN_AGGR_DIM], F32, tag="mv")
    nc.vector.bn_aggr(out=mv[:, :], in_=stats[:, :, :])
    rstd = pool.tile([p, 1], F32, tag="rstd")
    nc.scalar.activation(out=rstd[:, :], in_=mv[:, 1:2], func=AF.Sqrt,
                         bias=eps_tile[:p, :], scale=1.0)
    nc.vector.reciprocal(out=rstd[:, :], in_=rstd[:, :])
    return mv, rstd


def build_program():
    nc = bacc.Bacc(None, target_bir_lowering=False, num_devices=8)
    x = nc.declare_dram_parameter("x", [N, D], F32, isOutput=False)
    xs = nc.declare_dram_parameter("xs", [SLAB, D], F32, isOutput=False)
    wq = nc.declare_dram_parameter("wq", [D, G * HD], F32R, isOutput=False)
    wkv = nc.declare_dram_parameter("wkv", [D, 2 * HD], F32R, isOutput=False)
    wo = nc.declare_dram_parameter("wo", [G * HD, D], F32R, isOutput=False)
    g1b1 = nc.declare_dram_parameter("g1b1", [2, D], F32, isOutput=False)
    g2b2 = nc.declare_dram_parameter("g2b2", [2, D], F32, isOutput=False)
    mask = nc.declare_dram_parameter("mask", [P, P], F32, isOutput=False)
    y = nc.declare_dram_parameter("y", [SLAB, D], F32, isOutput=True)

    with tile.TileContext(nc) as tc, ExitStack() as ctx:
        const = ctx.enter_context(tc.tile_pool(name="const", bufs=1))
        big = ctx.enter_context(tc.tile_pool(name="big", bufs=1))
        work = ctx.enter_context(tc.tile_pool(name="work", bufs=3))
        stp = ctx.enter_context(tc.tile_pool(name="stats", bufs=4))
        outp = ctx.enter_context(tc.tile_pool(name="outp", bufs=3))
        pss = ctx.enter_context(tc.tile_pool(name="pss", bufs=4, space="PSUM"))
        pso = ctx.enter_context(tc.tile_pool(name="pso", bufs=1, space="PSUM"))
        dram = ctx.enter_context(tc.tile_pool(name="dram", bufs=1, space="DRAM"))

        # ---- constants ----
        ident = const.tile([P, P], F32)
        make_identity(nc, ident)
        identr = const.tile([P, P], F32R)
        nc.scalar.copy(out=identr[:, :], in_=ident[:, :])
        mask_sb = const.tile([P, P], F32)
        nc.sync.dma_start(out=mask_sb[:, :], in_=mask[:, :])
        eps_t = const.tile([P, 1], F32)
        nc.vector.memset(eps_t[:, :], EPS)
        ones01f = const.tile([1, HD], F32)
        nc.vector.memset(ones01f[:, :], 0.1)
        ones01 = const.tile([1, HD], F32R)
        nc.scalar.copy(out=ones01[:, :], in_=ones01f[:, :])  # 0.1 residual scale
        g1c = const.tile([P, DC], F32)
        b1c = const.tile([P, DC], F32)
        nc.sync.dma_start(out=g1c[:, :], in_=g1b1[0, :].rearrange("(c p) -> p c", p=P))
        nc.sync.dma_start(out=b1c[:, :], in_=g1b1[1, :].rearrange("(c p) -> p c", p=P))
        g2bc = const.tile([P, D], F32)
        b2bc = const.tile([P, D], F32)
        for row, dst in ((0, g2bc), (1, b2bc)):
            src = g2b2[row : row + 1, :]
            bsrc = bass.AP(tensor=src.tensor, offset=src.offset,
                           ap=[[0, P]] + src.ap[1:])
            nc.sync.dma_start(out=dst[:, :], in_=bsrc)

        # weights
        wq_sb = const.tile([P, DC, G * HD], F32R)
        nc.sync.dma_start(out=wq_sb[:, :, :],
                          in_=wq.rearrange("(c p) m -> p c m", p=P))
        wkv_sb = const.tile([P, DC, 2 * HD], F32R)
        nc.sync.dma_start(out=wkv_sb[:, :, :],
                          in_=wkv.rearrange("(c p) m -> p c m", p=P))
        wo_sb = const.tile([P, 2, D], F32R)
        nc.sync.dma_start(out=wo_sb[:, :, :],
                          in_=wo.rearrange("(m p) d -> p m d", p=P))

        # ---- big persistent tensors ----
        tpool = ctx.enter_context(tc.tile_pool(name="tpool", bufs=2))
        qT = big.tile([HD, G, N], F32R)      # Q^T per head, all at base partition 0
        kvT = big.tile([P, N], F32R)         # rows 0-63 K^T, 64-127 V^T
        vt1 = big.tile([P, NT, HD + 1], F32R)  # [V_j | ones] per key chunk
        attnT = big.tile([P, 2, N], F32R)    # normalized attn^T (256 x N)

        # ---- stage 1+2: LN1 + transpose + projections, per 512-token chunk ----
        for f in range(NC_):
            tT = tpool.tile([P, DC, FD], F32R, tag="tT")
            for it in range(FD // P):
                i = f * (FD // P) + it
                xt = work.tile([P, D], F32, tag="xt")
                nc.sync.dma_start(out=xt[:, :], in_=x[i * P : (i + 1) * P, :])
                mv, rstd = _ln_stats(nc, stp, xt[:, :], eps_t)
                nc.vector.tensor_scalar(out=xt[:, :], in0=xt[:, :],
                                        scalar1=mv[:, 0:1], scalar2=rstd[:, :],
                                        op0=ALU.subtract, op1=ALU.mult)
                for c in range(DC):
                    pt = pss.tile([P, FD], F32, tag="ps")
                    nc.tensor.transpose(pt[:, :P], xt[:, c * P : (c + 1) * P],
                                        ident[:, :])
                    nc.vector.tensor_scalar(out=tT[:, c, it * P : (it + 1) * P],
                                            in0=pt[:, :P],
                                            scalar1=g1c[:, c : c + 1],
                                            scalar2=b1c[:, c : c + 1],
                                            op0=ALU.mult, op1=ALU.add)
            # Q^T: psum rows 0-63 = head 2m, 64-127 = head 2m+1
            for m in range(2):
                ps = pss.tile([P, FD], F32, tag="ps")
                for c in range(DC):
                    nc.tensor.matmul(ps[:, :],
                                     wq_sb[:, c, m * P : (m + 1) * P],
                                     tT[:, c, :],
                                     start=(c == 0), stop=(c == DC - 1))
                nc.scalar.copy(out=qT[:, 2 * m, f * FD : (f + 1) * FD],
                               in_=ps[:HD, :])
                # upper half must land at base partition 0 -> SBUF bounce + DMA
                qtmp = work.tile([P, FD], F32R, tag="qtmp")
                nc.scalar.copy(out=qtmp[HD:P, :], in_=ps[HD:P, :])
                nc.sync.dma_start(out=qT[:, 2 * m + 1, f * FD : (f + 1) * FD],
                                  in_=qtmp[HD:P, :])
            ps = pss.tile([P, FD], F32, tag="ps")
            for c in range(DC):
                nc.tensor.matmul(ps[:, :], wkv_sb[:, c, :], tT[:, c, :],
                                 start=(c == 0), stop=(c == DC - 1))
            nc.scalar.copy(out=kvT[:, f * FD : (f + 1) * FD], in_=ps[:, :])

        # V^T -> V (per key chunk), plus the ones column
        onescol = const.tile([P, 1], F32)
        nc.vector.memset(onescol[:, :], 1.0)
        for j in range(NT):
            nc.scalar.copy(out=vt1[:, j, HD : HD + 1], in_=onescol[:, :])
            pt = pss.tile([P, FD], F32R, tag="ps")
            nc.tensor.transpose(pt[:, :HD], kvT[HD:P, j * P : (j + 1) * P],
                                identr[HD:P, HD:P])
            nc.scalar.copy(out=vt1[:, j, :HD], in_=pt[:, :HD])

        # ---- stage 3: attention, S^T orientation ----
        for h in range(G):
            qrow = (h % 2) * HD  # target row range inside attnT chunk h // 2
            qm = h // 2
            psO = pso.tile([HD + 1, N], F32)
            for c in range(NC_):
                c0 = c * FD
                for j in range(4 * c + 4):
                    q0 = max(c0, j * P)
                    w = (c + 1) * FD - q0
                    psS = pss.tile([P, FD], F32, tag="ps")
                    nc.tensor.matmul(psS[:, :w],
                                     kvT[0:HD, j * P : (j + 1) * P],
                                     qT[:, h, q0 : q0 + w],
                                     start=True, stop=True)
                    uT = work.tile([P, FD], F32R, tag="ut")
                    nc.scalar.activation(out=uT[:, :w], in_=psS[:, :w],
                                         func=AF.Exp, scale=SCALE)
                    if j // 4 == c:  # diagonal block -> causal mask
                        o = j * P - c0
                        nc.vector.tensor_mul(uT[:, o : o + P], uT[:, o : o + P],
                                             mask_sb[:, :])
                    nc.tensor.matmul(psO[:, q0 : q0 + w], vt1[:, j, :],
                                     uT[:, :w],
                                     start=(j == 0), stop=(j == 4 * c + 3))
                # normalize: attnT = psO[0:64] * (0.1 / Z)
                rz = stp.tile([1, FD], F32R, tag="rz")
                with nc.allow_low_precision(reason="1/Z in f32r feeds f32r matmul"):
                    nc.vector.reciprocal(out=rz[:, :],
                                         in_=psO[HD : HD + 1, c0 : c0 + FD])
                psB = pss.tile([P, FD], F32, tag="ps")
                nc.tensor.matmul(psB[:HD, :], ones01[:, :], rz[:, :],
                                 start=True, stop=True)
                bz = work.tile([HD, FD], F32, tag="bz")
                nc.scalar.copy(out=bz[:, :], in_=psB[:HD, :])
                if qrow == 0:
                    nc.vector.tensor_mul(attnT[0:HD, qm, c0 : c0 + FD],
                                         psO[0:HD, c0 : c0 + FD], bz[:, :])
                else:
                    at = work.tile([HD, FD], F32R, tag="atmp")
                    nc.vector.tensor_mul(at[:, :], psO[0:HD, c0 : c0 + FD],
                                         bz[:, :])
                    nc.sync.dma_start(out=attnT[HD:P, qm, c0 : c0 + FD],
                                      in_=at[:, :])

        # ---- stage 4: Wo partial product -> DRAM ----
        part = dram.tile([N, D], F32)
        rs = dram.tile([SLAB, D], F32)
        for i in range(NT):
            op = outp.tile([P, D], F32, tag="op")
            for f in range(2):
                ps = pss.tile([P, FD], F32, tag="ps")
                for m in range(2):
                    nc.tensor.matmul(ps[:, :],
                                     attnT[:, m, i * P : (i + 1) * P],
                                     wo_sb[:, m, f * FD : (f + 1) * FD],
                                     start=(m == 0), stop=(m == 1))
                nc.scalar.copy(out=op[:, f * FD : (f + 1) * FD], in_=ps[:, :])
            nc.sync.dma_start(out=part[i * P : (i + 1) * P, :], in_=op[:, :])

        # ---- stage 5: reduce-scatter + residual + LN2 ----
        nc.gpsimd.collective_compute(
            "ReduceScatter", ALU.add, replica_groups=RG,
            ins=[part[:, :]], outs=[rs[:, :]],
        )
        for t in range(ST):
            rt = work.tile([P, D], F32, tag="xt")
            nc.sync.dma_start(out=rt[:, :], in_=rs[t * P : (t + 1) * P, :])
            xt = work.tile([P, D], F32, tag="xt")
            nc.sync.dma_start(out=xt[:, :], in_=xs[t * P : (t + 1) * P, :])
            nc.vector.tensor_add(rt[:, :], rt[:, :], xt[:, :])
            mv, rstd = _ln_stats(nc, stp, rt[:, :], eps_t)
            nc.vector.tensor_scalar(out=rt[:, :], in0=rt[:, :],
                                    scalar1=mv[:, 0:1], scalar2=rstd[:, :],
                                    op0=ALU.subtract, op1=ALU.mult)
            nc.vector.tensor_mul(rt[:, :], rt[:, :], g2bc[:, :])
            nc.vector.tensor_add(rt[:, :], rt[:, :], b2bc[:, :])
            nc.sync.dma_start(out=y[t * P : (t + 1) * P, :], in_=rt[:, :])

    nc.finalize()
    return nc


_NC_CACHE = {}


def _get_program():
    if "nc" not in _NC_CACHE:
        _NC_CACHE["nc"] = build_program()
    return _NC_CACHE["nc"]


def make_in_maps(tokens, Wq, Wk, Wv, Wo, g1, b1, g2, b2):
    tokens = np.ascontiguousarray(tokens, np.float32)
    mask = np.triu(np.ones((P, P), np.float32))  # [key, query]: key <= query
    g1b1 = np.stack([np.asarray(g1, np.float32), np.asarray(b1, np.float32)])
    g2b2 = np.stack([np.asarray(g2, np.float32), np.asarray(b2, np.float32)])
    in_maps = []
    for cid in range(8):
        b, k = cid // 4, cid % 4
        r = cid % 4
        in_maps.append({
            "x": tokens[b],
            "xs": tokens[b][r * SLAB : (r + 1) * SLAB],
            "wq": np.ascontiguousarray(Wq[:, k * G * HD : (k + 1) * G * HD], np.float32),
            "wkv": np.ascontiguousarray(
                np.concatenate([Wk[:, k * HD : (k + 1) * HD],
                                Wv[:, k * HD : (k + 1) * HD]], axis=1), np.float32),
            "wo": np.ascontiguousarray(Wo[k * G * HD : (k + 1) * G * HD, :], np.float32),
            "g1b1": g1b1, "g2b2": g2b2, "mask": mask,
        })
    return in_maps


def kernel(tokens, Wq, Wk, Wv, Wo, g1, b1, g2, b2, _trace=False, _trace_kwargs=None):
    nc = _get_program()
    in_maps = make_in_maps(tokens, Wq, Wk, Wv, Wo, g1, b1, g2, b2)
    res = run_bass_kernel_spmd(nc, in_maps, list(range(8)),
                               trace=_trace, **(_trace_kwargs or {}))
    out = np.empty((B, N, D), np.float32)
    for cid in range(8):
        b, r = cid // 4, cid % 4
        out[b, r * SLAB : (r + 1) * SLAB] = res.results[cid]["y"]
    if _trace:
        return out, res
    return out



# revision 34
# speedup vs baseline: 1.2789x; 1.2789x over previous
"""GQA transformer block on 8 TRN2 cores — fp8 DoubleRow version.

Sharding: core = (b, k), b = batch (2), k = kv-head (4).

Per core:
- Host supplies raw x^T in fp8 plus prescaled (x64) fp8 weights.
- LayerNorm1 is folded into the projections: R = x @ (g1*W*64) computed
  directly from raw x^T; per-token mean/rstd come from Sum(x), Sum(x^2)
  matmuls against a ones column; the affine fixup
  Qhat = rstd*R - (mu*rstd)*cg + cb runs on DVE/gpsimd.
- Attention in S^T orientation (keys on partitions, 4 heads packed in the
  free dim), fp8 DoubleRow matmuls (2 contraction tiles per instruction),
  exp on the scalar engine over 2-bank psum pairs, causal mask via
  gpsimd affine_select on the diagonal blocks.
- Each core multiplies its 4 heads' attnT into its Wo row-slab (fp8
  DoubleRow) for all 2048 tokens, converts the partial to bf16 (4MB
  instead of the 8MB f32 of the old kernel) and ReduceScatters it over
  the 4 cores of the batch; the 512-row shard gets residual + LayerNorm2.
"""

import os
import sys
from contextlib import ExitStack

for _p in ("/opt/trn_rl_repo", "/root/.axon_site/_ro/trn_rl_repo"):
    if os.path.isdir(_p) and _p not in sys.path:
        sys.path.insert(0, _p)

import numpy as np
import ml_dtypes

import concourse.bass as bass
import concourse.bacc as bacc
import concourse.tile as tile
from concourse import mybir
from concourse.bass_utils import run_bass_kernel_spmd
from concourse.masks import make_identity

B, N, D = 2, 2048, 1024
HQ, HKV, HD = 16, 4, 64
G = HQ // HKV
EPS = 1e-5
P = 128
NT = N // P          # 16 token tiles
DC = D // P          # 8 d-chunks
SLAB = N // 4        # 512 rows per core for the output slab
ST = SLAB // P       # 4 token tiles per slab
F32 = mybir.dt.float32
BF16 = mybir.dt.bfloat16
F8 = mybir.dt.float8e4
FD = 512             # free-dim chunk (one PSUM bank)
NC_ = N // FD        # 4 free chunks
RG = [[0, 1, 2, 3], [4, 5, 6, 7]]
AF = mybir.ActivationFunctionType
ALU = mybir.AluOpType
PM = mybir.MatmulPerfMode
WS = 64.0                      # fp8 weight prescale
K2 = (1.0 / np.sqrt(HD)) / (WS * WS)   # exp scale on raw score psum
RZV = 1.0 / WS                 # folds V'' = 64*V back out during normalize
YS = 0.1 / WS                  # output scale on the Wo psum
SX_ROW, SX2_ROW = 0, 64        # rows in the stats psum tile
F8NP = ml_dtypes.float8_e4m3fn


def build_program():
    nc = bacc.Bacc(None, target_bir_lowering=False, num_devices=8)
    xt = nc.declare_dram_parameter("xt", [D, N], F8, isOutput=False)
    xs = nc.declare_dram_parameter("xs", [SLAB, D], F32, isOutput=False)
    wq = nc.declare_dram_parameter("wq", [D, G * HD], F8, isOutput=False)
    wkv = nc.declare_dram_parameter("wkv", [D, 2 * HD], F8, isOutput=False)
    wo = nc.declare_dram_parameter("wo", [HD, G, D], F8, isOutput=False)
    cgb = nc.declare_dram_parameter("cgb", [2, 3 * P], F32, isOutput=False)
    g2b2 = nc.declare_dram_parameter("g2b2", [2, D], F32, isOutput=False)
    y = nc.declare_dram_parameter("y", [SLAB, D], F32, isOutput=True)

    with tile.TileContext(nc) as tc, ExitStack() as ctx:
        const = ctx.enter_context(tc.tile_pool(name="const", bufs=1))
        big = ctx.enter_context(tc.tile_pool(name="big", bufs=1))
        work = ctx.enter_context(tc.tile_pool(name="work", bufs=3))
        stp = ctx.enter_context(tc.tile_pool(name="stats", bufs=4))
        utp = ctx.enter_context(tc.tile_pool(name="utp", bufs=3))
        bcp = ctx.enter_context(tc.tile_pool(name="bcp", bufs=2))
        pssh = ctx.enter_context(tc.tile_pool(name="pssh", bufs=2, space="PSUM"))
        pssc = ctx.enter_context(tc.tile_pool(name="pssc", bufs=2, space="PSUM"))
        psoo = ctx.enter_context(tc.tile_pool(name="psoo", bufs=1, space="PSUM"))
        pstp = ctx.enter_context(tc.tile_pool(name="pstp", bufs=1, space="PSUM"))
        dram = ctx.enter_context(tc.tile_pool(name="dram", bufs=1, space="DRAM"))

        # ---- constants ----
        ident = const.tile([P, P], F32)
        make_identity(nc, ident)
        identb = const.tile([P, P], BF16)
        nc.scalar.copy(out=identb[:, :], in_=ident[:, :])
        eps_t = const.tile([1, 1], F32)
        nc.vector.memset(eps_t[:, :], EPS)
        eps_p = const.tile([P, 1], F32)
        nc.vector.memset(eps_p[:, :], EPS)
        # dual-fp8 ldweights needs >=64 stationary columns per k-tile:
        # use a 64-wide block whose first column is ones (rest zero)
        ones8 = const.tile([P, DC, HD], F8)
        nc.vector.memset(ones8[:, :, :], 0.0)
        nc.vector.memset(ones8[:, :, 0:1], 1.0)
        ncgq = const.tile([P, 3], F32)
        cbq = const.tile([P, 3], F32)
        nc.sync.dma_start(out=ncgq[:, :], in_=cgb[0, :].rearrange("(c p) -> p c", p=P))
        nc.sync.dma_start(out=cbq[:, :], in_=cgb[1, :].rearrange("(c p) -> p c", p=P))
        g2bc = const.tile([P, D], F32)
        b2bc = const.tile([P, D], F32)
        for row, dst in ((0, g2bc), (1, b2bc)):
            src = g2b2[row : row + 1, :]
            bsrc = bass.AP(tensor=src.tensor, offset=src.offset,
                           ap=[[0, P]] + src.ap[1:])
            nc.sync.dma_start(out=dst[:, :], in_=bsrc)

        # weights
        wq_sb = const.tile([P, DC, G * HD], F8)
        nc.sync.dma_start(out=wq_sb[:, :, :],
                          in_=wq.rearrange("(c p) m -> p c m", p=P))
        wkv_sb = const.tile([P, DC, 2 * HD], F8)
        nc.sync.dma_start(out=wkv_sb[:, :, :],
                          in_=wkv.rearrange("(c p) m -> p c m", p=P))
        wo_sb = const.tile([HD, G, D], F8)
        nc.sync.dma_start(out=wo_sb[:, :, :], in_=wo[:, :, :])

        # big persistent tensors
        xt_sb = big.tile([P, DC, N], F8)
        nc.sync.dma_start(out=xt_sb[:, :, :],
                          in_=xt.rearrange("(c p) n -> p c n", p=P))
        qT = big.tile([32, 2, G, NT, P], F8)   # [hd-half, kt, head, qtile, q]
        kT = big.tile([32, 2, NT, P], F8)      # [hd-half, kt, keyblock, key]
        vt1 = big.tile([P, NT // 2, 2, P], F8)  # [key, jpair, kt, hd|WS|pad]
        atm = big.tile([HD, G, N], F8)         # my normalized attn^T
        xs_sb = big.tile([P, ST, D], F32)
        nc.sync.dma_start(out=xs_sb[:, :, :],
                          in_=xs.rearrange("(t p) d -> p t d", p=P))

        # ones column carries WS so psO's Z row is WS*Z and 1/(WS*Z) folds
        # the V'' descale into the normalize multiply for free; pad columns
        # 65..127 are zero (dual-fp8 ldweights needs >=64-wide stationary)
        nc.vector.memset(vt1[:, :, :, :], 0.0)
        nc.vector.memset(vt1[:, :, :, HD : HD + 1], WS)
        zrow = dram.tile([NT, FD], BF16)

        part = dram.tile([N, D], BF16)
        rs = dram.tile([SLAB, D], BF16)

        # ================= per 512-token chunk =================
        for f in range(NC_):
            fsl = slice(f * FD, (f + 1) * FD)
            # ---- stats: Sum(x), Sum(x^2) over d ----
            x2 = work.tile([P, DC, FD], F8, tag="x2")
            nc.vector.tensor_mul(x2[:, :, :], xt_sb[:, :, fsl], xt_sb[:, :, fsl])
            pstat = pssh.tile([P, FD], F32, tag="ps")
            for c in range(DC // 2):
                nc.tensor.matmul(pstat[0:HD, :],
                                 ones8[:, 2 * c : 2 * c + 2, :],
                                 xt_sb[:, 2 * c : 2 * c + 2, fsl],
                                 start=(c == 0), stop=(c == DC // 2 - 1),
                                 perf_mode=PM.DoubleRow)
            pstat2 = pssh.tile([P, FD], F32, tag="ps")
            for c in range(DC // 2):
                nc.tensor.matmul(pstat2[0:HD, :],
                                 ones8[:, 2 * c : 2 * c + 2, :],
                                 x2[:, 2 * c : 2 * c + 2, :],
                                 start=(c == 0), stop=(c == DC // 2 - 1),
                                 perf_mode=PM.DoubleRow)
            murow = stp.tile([1, FD], F32, tag="mu")
            nc.vector.tensor_scalar(out=murow[:, :],
                                    in0=pstat[0:1, :],
                                    scalar1=1.0 / D, scalar2=None,
                                    op0=ALU.mult)
            varrow = stp.tile([1, FD], F32, tag="var")
            nc.vector.tensor_scalar(out=varrow[:, :],
                                    in0=pstat2[0:1, :],
                                    scalar1=1.0 / D, scalar2=None,
                                    op0=ALU.mult)
            mu2 = stp.tile([1, FD], F32, tag="mu2")
            nc.vector.tensor_mul(mu2[:, :], murow[:, :], murow[:, :])
            nc.vector.tensor_sub(varrow[:, :], varrow[:, :], mu2[:, :])
            rstdr = stp.tile([1, FD], F32, tag="rstd")
            nc.scalar.activation(out=rstdr[:, :], in_=varrow[:, :],
                                 func=AF.Sqrt, bias=eps_t[:, :], scale=1.0)
            nc.vector.reciprocal(out=rstdr[:, :], in_=rstdr[:, :])
            murr = stp.tile([1, FD], F32, tag="murr")
            nc.vector.tensor_mul(murr[:, :], murow[:, :], rstdr[:, :])
            # broadcast to 128 partitions: bounce through DRAM, then
            # 0-stride partition read (only legal from DRAM sources)
            srow = dram.tile([2, FD], F32, tag=f"srow{f}")
            nc.sync.dma_start(out=srow[0:1, :], in_=rstdr[:, :])
            nc.sync.dma_start(out=srow[1:2, :], in_=murr[:, :])
            rstdB = bcp.tile([P, FD], F32, tag="rstdB")
            murB = bcp.tile([P, FD], F32, tag="murB")
            for dst, row in ((rstdB, 0), (murB, 1)):
                s = srow[row : row + 1, :]
                bsrc = bass.AP(tensor=s.tensor, offset=s.offset,
                               ap=[[0, P]] + s.ap[1:])
                nc.sync.dma_start(out=dst[:, :], in_=bsrc)

            # ---- projections + LN fixup ----
            for m in range(3):
                ps = pssh.tile([P, FD], F32, tag="ps")
                wsrc = wq_sb if m < 2 else wkv_sb
                msl = slice((m % 2) * P, (m % 2) * P + P) if m < 2 else slice(0, P)
                for c in range(DC // 2):
                    nc.tensor.matmul(ps[:, :],
                                     wsrc[:, 2 * c : 2 * c + 2, msl],
                                     xt_sb[:, 2 * c : 2 * c + 2, fsl],
                                     start=(c == 0), stop=(c == DC // 2 - 1),
                                     perf_mode=PM.DoubleRow)
                # A = cb - murstd*cg  (ncg pre-negated on host)
                at = work.tile([P, FD], F32, tag="at")
                nc.gpsimd.tensor_scalar(out=at[:, :], in0=murB[:, :],
                                        scalar1=ncgq[:, m : m + 1],
                                        scalar2=cbq[:, m : m + 1],
                                        op0=ALU.mult, op1=ALU.add)
                bt = work.tile([P, FD], F32, tag="bt")
                nc.vector.tensor_mul(bt[:, :], ps[:, :], rstdB[:, :])
                if m < 2:
                    qstage = work.tile([P, FD], F8, tag="qs")
                    nc.vector.tensor_add(qstage[:, :], bt[:, :], at[:, :])
                    for hh in range(2):
                        h = 2 * m + hh
                        for kt in range(2):
                            r0 = hh * HD + kt * 32
                            nc.sync.dma_start(
                                out=qT[:, kt, h, 4 * f : 4 * f + 4, :],
                                in_=qstage[r0 : r0 + 32, :])
                else:
                    kstage = work.tile([P, FD], F8, tag="qs")
                    nc.vector.tensor_add(kstage[0:HD, :], bt[0:HD, :],
                                         at[0:HD, :])
                    for kt in range(2):
                        nc.sync.dma_start(out=kT[:, kt, 4 * f : 4 * f + 4, :],
                                          in_=kstage[kt * 32 : kt * 32 + 32, :])
                    vstage = work.tile([P, FD], BF16, tag="vs")
                    nc.vector.tensor_add(vstage[HD:P, :], bt[HD:P, :],
                                         at[HD:P, :])
                    for jj in range(4):
                        j = 4 * f + jj
                        pt = pstp.tile([P, P], BF16, tag="pt")
                        nc.tensor.transpose(
                            pt[:, :HD],
                            vstage[HD:P, jj * P : (jj + 1) * P],
                            identb[HD:P, HD:P])
                        nc.vector.tensor_scalar(
                            out=vt1[:, j // 2, j % 2, :HD],
                            in0=pt[:, :HD], scalar1=1.0, scalar2=None,
                            op0=ALU.mult)

            # ---- attention for query tiles of this chunk ----
            for it in range(4):
                i = 4 * f + it
                isl = slice(i * P, (i + 1) * P)
                psO = psoo.tile([P, FD], F32, tag="psO")
                npair = (i + 1) // 2
                single = (i % 2 == 0)
                for p_ in range(npair + (1 if single else 0)):
                    lastp = (p_ == npair - (0 if single else 1)) and not single
                    psS = pssc.tile([P, 2, FD], F32, tag="psS")
                    uT = utp.tile([P, 2, FD], F8, tag="uT")
                    if p_ < npair:
                        for t in range(2):
                            j = 2 * p_ + t
                            nc.tensor.matmul(psS[:, t, :],
                                             kT[:, :, j, :],
                                             qT[:, :, :, i, :],
                                             start=True, stop=True,
                                             perf_mode=PM.DoubleRow)
                        nc.scalar.activation(out=uT[:, :, :], in_=psS[:, :, :],
                                             func=AF.Exp, scale=K2)
                        if 2 * p_ + 1 == i:
                            nc.gpsimd.affine_select(
                                out=uT[:, 1, :].rearrange("p (h q) -> p h q", h=G),
                                in_=uT[:, 1, :].rearrange("p (h q) -> p h q", h=G),
                                pattern=[[0, G], [1, P]],
                                channel_multiplier=-1, base=0,
                                compare_op=ALU.is_ge, fill=0.0)
                        nc.tensor.matmul(psO[:, :], vt1[:, p_, :, :],
                                         uT[:, :, :],
                                         start=(p_ == 0),
                                         stop=lastp,
                                         perf_mode=PM.DoubleRow)
                    else:
                        # odd singleton: block j == i (diagonal)
                        nc.tensor.matmul(psS[:, 0, :],
                                         kT[:, :, i, :],
                                         qT[:, :, :, i, :],
                                         start=True, stop=True,
                                         perf_mode=PM.DoubleRow)
                        nc.scalar.activation(out=uT[:, 0, :],
                                             in_=psS[:, 0, :],
                                             func=AF.Exp, scale=K2)
                        nc.gpsimd.affine_select(
                            out=uT[:, 0, :].rearrange("p (h q) -> p h q", h=G),
                            in_=uT[:, 0, :].rearrange("p (h q) -> p h q", h=G),
                            pattern=[[0, G], [1, P]],
                            channel_multiplier=-1, base=0,
                            compare_op=ALU.is_ge, fill=0.0)
                        nc.tensor.matmul(psO[:, :], vt1[:, i // 2, 0, :],
                                         uT[:, 0, :],
                                         start=(i == 0), stop=True)
                # normalize: attnT = psO[0:64] * (1 / (WS*Z))
                rz = stp.tile([1, FD], BF16, tag="rz")
                with nc.allow_low_precision(reason="1/Z attn normalize"):
                    nc.vector.reciprocal(out=rz[:, :],
                                         in_=psO[HD : HD + 1, :])
                nc.sync.dma_start(out=zrow[i : i + 1, :], in_=rz[:, :])
                bzs = bcp.tile([HD, FD], BF16, tag="bzs")
                s = zrow[i : i + 1, :]
                bsrc = bass.AP(tensor=s.tensor, offset=s.offset,
                               ap=[[0, HD]] + s.ap[1:])
                nc.sync.dma_start(out=bzs[:, :], in_=bsrc)
                nc.vector.tensor_mul(
                    atm[:, :, isl],
                    psO[0:HD, :].rearrange("p (h q) -> p h q", h=G),
                    bzs[:, :].rearrange("p (h q) -> p h q", h=G))
            # ---- Wo partial for this chunk's 4 token tiles ----
            for it in range(4):
                i = 4 * f + it
                psW = pssc.tile([P, 2, FD], F32, tag="psS")
                for fh in range(2):
                    for p_ in range(G // 2):
                        nc.tensor.matmul(
                            psW[:, fh, :],
                            atm[:, 2 * p_ : 2 * p_ + 2, i * P : (i + 1) * P],
                            wo_sb[:, 2 * p_ : 2 * p_ + 2,
                                  fh * FD : (fh + 1) * FD],
                            start=(p_ == 0), stop=(p_ == G // 2 - 1),
                            perf_mode=PM.DoubleRow)
                pstage = work.tile([P, D], BF16, tag="pst")
                if i % 2 == 0:
                    nc.scalar.activation(
                        out=pstage[:, :].rearrange("p (a b) -> p a b", a=2),
                        in_=psW[:, :, :], func=AF.Copy, scale=1.0)
                else:
                    nc.vector.tensor_scalar(
                        out=pstage[:, :].rearrange("p (a b) -> p a b", a=2),
                        in0=psW[:, :, :], scalar1=1.0, scalar2=None,
                        op0=ALU.mult)
                nc.sync.dma_start(out=part[i * P : (i + 1) * P, :],
                                  in_=pstage[:, :])

        # ================= exchange + output =================
        nc.gpsimd.collective_compute(
            "ReduceScatter", ALU.add, replica_groups=RG,
            ins=[part[:, :]], outs=[rs[:, :]],
        )
        for t in range(ST):
            rt = work.tile([P, D], F32, tag="rt")
            rsb = work.tile([P, D], BF16, tag="rsb")
            nc.sync.dma_start(out=rsb[:, :], in_=rs[t * P : (t + 1) * P, :])
            nc.vector.tensor_scalar(out=rt[:, :], in0=rsb[:, :],
                                    scalar1=YS, scalar2=None, op0=ALU.mult)
            nc.vector.tensor_add(rt[:, :], rt[:, :], xs_sb[:, t, :])
            # LN2
            stats = stp.tile([P, 2, nc.vector.BN_STATS_DIM], F32, tag="bst")
            xg = rt[:, :].rearrange("p (s d) -> p s d", s=2)
            for s in range(2):
                nc.vector.bn_stats(out=stats[:, s, :], in_=xg[:, s, :])
            mv = stp.tile([P, nc.vector.BN_AGGR_DIM], F32, tag="mv")
            nc.vector.bn_aggr(out=mv[:, :], in_=stats[:, :, :])
            rstd2 = stp.tile([P, 1], F32, tag="r2")
            nc.scalar.activation(out=rstd2[:, :], in_=mv[:, 1:2], func=AF.Sqrt,
                                 bias=eps_p[:, :], scale=1.0)
            nc.vector.reciprocal(out=rstd2[:, :], in_=rstd2[:, :])
            nc.vector.tensor_scalar(out=rt[:, :], in0=rt[:, :],
                                    scalar1=mv[:, 0:1], scalar2=rstd2[:, :],
                                    op0=ALU.subtract, op1=ALU.mult)
            nc.gpsimd.tensor_mul(rt[:, :], rt[:, :], g2bc[:, :])
            nc.gpsimd.tensor_add(rt[:, :], rt[:, :], b2bc[:, :])
            nc.sync.dma_start(out=y[t * P : (t + 1) * P, :], in_=rt[:, :])

    nc.finalize()
    return nc


_NC_CACHE = {}


def _get_program():
    if "nc" not in _NC_CACHE:
        _NC_CACHE["nc"] = build_program()
    return _NC_CACHE["nc"]


def make_in_maps(tokens, Wq, Wk, Wv, Wo, g1, b1, g2, b2):
    tokens = np.asarray(tokens, np.float32)
    g1 = np.asarray(g1, np.float32)
    b1 = np.asarray(b1, np.float32)
    g2b2 = np.stack([np.asarray(g2, np.float32), np.asarray(b2, np.float32)])
    in_maps = []
    for cid in range(8):
        b, k = cid // 4, cid % 4
        r = k
        wqk = np.asarray(Wq, np.float32)[:, k * G * HD : (k + 1) * G * HD]
        wkk = np.asarray(Wk, np.float32)[:, k * HD : (k + 1) * HD]
        wvk = np.asarray(Wv, np.float32)[:, k * HD : (k + 1) * HD]
        wcat = np.concatenate([wqk, wkk, wvk], axis=1)  # [D, 384]
        cg = WS * (g1[:, None] * wcat).sum(0)
        cb = WS * (b1[:, None] * wcat).sum(0)
        cgb = np.stack([-cg, cb]).astype(np.float32)
        wo8 = np.ascontiguousarray(
            (np.asarray(Wo, np.float32)[k * G * HD : (k + 1) * G * HD] * WS)
            .reshape(G, HD, D).transpose(1, 0, 2)).astype(F8NP)
        in_maps.append({
            "xt": np.ascontiguousarray(tokens[b].T).astype(F8NP),
            "xs": np.ascontiguousarray(tokens[b][r * SLAB : (r + 1) * SLAB]),
            "wq": np.ascontiguousarray(WS * g1[:, None] * wqk).astype(F8NP),
            "wkv": np.ascontiguousarray(
                WS * g1[:, None] * np.concatenate([wkk, wvk], 1)).astype(F8NP),
            "wo": wo8,
            "cgb": cgb,
            "g2b2": g2b2,
        })
    return in_maps


def kernel(tokens, Wq, Wk, Wv, Wo, g1, b1, g2, b2, _trace=False, _trace_kwargs=None):
    nc = _get_program()
    in_maps = make_in_maps(tokens, Wq, Wk, Wv, Wo, g1, b1, g2, b2)
    res = run_bass_kernel_spmd(nc, in_maps, list(range(8)),
                               trace=_trace, **(_trace_kwargs or {}))
    out = np.empty((B, N, D), np.float32)
    for cid in range(8):
        b, r = cid // 4, cid % 4
        out[b, r * SLAB : (r + 1) * SLAB] = res.results[cid]["y"]
    if _trace:
        return out, res
    return out
